# revision 32
# baseline (speedup 1.0000x reference)
"""AttentionBlock kernel for Trainium2 (Bass/Tile), data-parallel over batch.

Reference computation (per batch b of 8, N = H*W = 4096, C = 256):
    q = x @ wq + bq ; k = x @ wk + bk ; v = x @ wv + bv          [N, C]
    s = (q @ k^T) / sqrt(C)                                      [N, N]
    a = softmax(s, axis=-1)
    o = a @ v                                                    [N, C]
    out = x + o @ wp + bp                                        [N, C]

Sharding: one batch per NeuronCore (8 batches, 8 cores), no collectives.

Per-core layout strategy ("S^T layout" - no attention transposes):
  - x is loaded naturally [n, c] and PE-transposed once to xT [c, n].
  - qT, kT [c, n] computed with weights as stationary operands.
  - v [n, c] computed naturally (xT slices stationary).
  - For each query block of 512 columns:
      for each key chunk m (32 chunks of 128 rows):
        sT[m-chunk]   = kT-slice.T @ qT-block      (PSUM [128k, 512q])
        eT = exp(sT / 16)                          (ACT, PSUM->SBUF)
        rawT[c-chunk] += v-slice.T @ eT            (PSUM [128c, 512q], accum)
        colsum += eT                               (DVE, SBUF accum)
      denom[q-sub] = colsum-slice.T @ ones         (partition reduce, [128q, 1])
      recip = 1/denom                              (DVE)
      proj[q-sub] = rawT-slice.T @ wp              (natural [128q, 256c])
      out = proj * recip + (x + bp + bv @ wp)      (ACT scale + DVE add)
  The softmax denominator division is deferred: it commutes with the wp
  contraction because it is a per-query scaling. bv also commutes through
  (attention rows sum to 1), folded into an effective output bias.

Big matmul operands (xT, qT, kT, v, e, rawT, weights) are BF16 (PSUM
accumulation stays fp32; end-to-end rel err ~7.6e-3 vs the 2e-2 gate).
FP8/DoubleRow was evaluated numerically and blows the error gate (6.6e-2
for fp8 q/k alone) - e4m3's 3 mantissa bits are ~32x coarser than bf16.

Schedule (best measured 291.2 us vs 391.3 us baseline; rel err
7.6e-3): targets zero PE stalls at full clock (2.4 GHz; the chip's P0
power state sometimes pins it at 2.0 GHz across runs - all structure
below helps at either clock):
  - The raw (v.T @ eT) pair for key-chunk mk issues TWO slots behind the
    score pair for mk (three right after a block boundary), giving the
    ~690 ns exp activation a full slot of latency slack vs the ~864 ns
    slot. The original in-slot ordering stalled the PE ~370 ns every
    other chunk (~45 us total).
  - Per-query-block boundary work (denominator reduce, reciprocal, wp
    proj, epilogue) drains one piece every other slot into the next
    block's stream; rawT PSUM drains on ACT, epilogue mul/add on DVE,
    output rows leave as one DMA per block (per-row for the last block
    so the tail pipelines with the now-idle ACT doing the muls).
  - HAM warm-up: 8 identity matmuls flip the PE clock gate to 2.4 GHz
    right as x arrives; keep-warm matmuls are woven through the
    transpose-only head (PE transposes don't count as HAM activity).
  - Head: DMA issues cost ~750 ns each and are strictly ordered per
    queue, so they are split across the two HWDGE queues - x on Sync,
    weights/biases on ACT - and the weight casts run on ACT so DVE's
    transpose copies never gate stage_b. First PE work ~8 us vs 18 us.
  - Phase 1 runs stage_a (transposes) two nb ahead of stage_b (qkv
    projections); all of x stays resident in SBUF (32 KB/partition) and
    the epilogue residual add reads it in place.
  - The attention stream is a generator: its first 28 slots (query
    block 0, whose kT/v chunks come from already-emitted stage_b
    groups) are pumped into phase 1, four after each stage_b, so x-DMA
    and copy waits are filled with ready matmul work and the kernel is
    one continuous PE stream. Pumping deeper than 4*nb+4 cumulative
    slots would read kT chunks emitted LATER in PE program order - an
    in-order-queue deadlock - so 4/group is the safe maximum.
"""

import numpy as np

import concourse.mybir as mybir
import concourse.tile as tile
from concourse import bacc
from concourse import bass_utils
from concourse.masks import make_identity

# Problem shape (hardcoded per contract).
B, H, W, C = 8, 64, 64, 256
N = H * W  # 4096
P = 128
C2 = C // P  # 2 chunks of input/output channels
NK = N // P  # 32 key chunks
QB = 512  # query block width (free dim of S^T matmuls)
NQB = N // QB  # 8 query blocks
QSUB = QB // P  # 4 query sub-blocks of 128 per block
SCALE = float(C) ** -0.5  # 1/16

F32 = mybir.dt.float32
F32R = mybir.dt.float32r
BF16 = mybir.dt.bfloat16
AF = mybir.ActivationFunctionType

_CACHED_NC = None


def _build():
    nc = bacc.Bacc("TRN2", target_bir_lowering=False, debug=False)

    x_d = nc.dram_tensor("x", [N, C], F32, kind="ExternalInput").ap()
    w_d = {
        name: nc.dram_tensor(name, [C, C], F32, kind="ExternalInput").ap()
        for name in ("wq", "wk", "wv", "wp")
    }
    b_d = {
        name: nc.dram_tensor(name, [C], F32, kind="ExternalInput").ap()
        for name in ("bq", "bk", "bv", "bp")
    }
    out_d = nc.dram_tensor("out", [N, C], F32, kind="ExternalOutput").ap()

    with tile.TileContext(nc) as tc:
        _emit(nc, tc, x_d, w_d, b_d, out_d)
    nc.compile()
    return nc


def _emit(nc, tc, x_d, w_d, b_d, out_d):
    import contextlib

    ctx = contextlib.ExitStack()
    with ctx:
        consts = ctx.enter_context(tc.tile_pool(name="consts", bufs=1))
        big = ctx.enter_context(tc.tile_pool(name="big", bufs=1))
        # x stays resident for the whole kernel: 8 nb tiles of
        # [128, 4, 256] fp32 (epilogue residual reads them in place).
        xload = ctx.enter_context(tc.tile_pool(name="xload", bufs=8))
        # e_t ring depth 8: exp(mk) WAR-waits colsum(mk-depth) on DVE
        # (colsum measures ~690 ns/slot, and DVE lags ~2 us through the
        # boundary-extras stretch); depth 6 still let that lag stall the
        # score stream ~850 ns twice per query block.
        exp_pool = ctx.enter_context(tc.tile_pool(name="exp", bufs=8))
        sums = ctx.enter_context(tc.tile_pool(name="sums", bufs=2))
        rawsb = ctx.enter_context(tc.tile_pool(name="rawsb", bufs=2))
        epil = ctx.enter_context(tc.tile_pool(name="epil", bufs=4))

        # Static PSUM pools, 8 banks: st 2 + raw 2 + proj(v/wp outs) 2 +
        # misc (transposes, denominators, bias prep) 2.
        ps_st = ctx.enter_context(tc.tile_pool(name="ps_st", bufs=2, space="PSUM"))
        ps_raw = ctx.enter_context(tc.tile_pool(name="ps_raw", bufs=1, space="PSUM"))
        ps_proj = ctx.enter_context(tc.tile_pool(name="ps_proj", bufs=2, space="PSUM"))
        ps_misc = ctx.enter_context(tc.tile_pool(name="ps_misc", bufs=2, space="PSUM"))

        # ---- constants + DMA issues -----------------------------------
        # DMA issues cost ~750 ns each on the issuing engine and are
        # strictly ordered per queue: x loads go on Sync, weights and
        # bias rows on ACT (both are HWDGE engines) so the chains overlap.
        identity = consts.tile([P, P], F32)
        make_identity(nc, identity[:])

        # PE warm-up: the HAM clock gate holds the PE at 1.2 GHz until it
        # has seen ~3.4 us of sustained matmul activity, and transpose-mode
        # instructions don't count. The first real matmuls land ~11 us in
        # (x DMA latency) - burn ~18 fp32 matmuls on the identity while
        # waiting so the real stream starts at 2.4 GHz.
        # 8 matmuls x ~420 ns cold = ~3.4 us: flips HAM right as the
        # first x tile lands (~11 us) without delaying the transposes
        # (18 here measured as a net loss - they pushed phase 1 out).
        for wi in range(8):
            wps = ps_misc.tile([P, P], F32, tag="misc", name=f"warm_{wi}")
            nc.tensor.matmul(wps[:], identity[:], identity[:], start=True, stop=True)

        def warm_trickle(wi):
            # one keep-warm matmul (own PSUM ring slot in the idle score
            # pool, so it never steals the transposes' misc/rawT slots)
            wps = ps_st.tile([P, P], F32, tag="st", name=f"wt_{wi}")
            nc.tensor.matmul(wps[:], identity[:], identity[:], start=True, stop=True)

        x_tiles = {}  # nb -> [P, 4, C] fp32 tile (rows nb*512 .. +511)
        for nb in range(NQB):
            xt = xload.tile([P, 4, C], F32, tag="x_in", name=f"x_in_{nb}")
            n0 = nb * QB
            nc.sync.dma_start(
                xt[:], x_d[n0 : n0 + QB, :].rearrange("(t p) c -> p t c", p=P)
            )
            x_tiles[nb] = xt

        ones_col = consts.tile([P, 1], F32)
        nc.vector.memset(ones_col[:], 1.0)
        ones_row = consts.tile([1, P], F32)
        nc.vector.memset(ones_row[:], 1.0)

        # Weights: [C, C] -> [P, C2, C] (ci = c2*128 + p on partitions),
        # issued on the ACT queue, wv first (stage_b(0) needs it first).
        w_stage = {}
        w_sb = {}
        for name in ("wv", "wq", "wk", "wp"):
            w_stage[name] = consts.tile(
                [P, C2, C], F32, tag=f"ws_{name}", name=f"ws_{name}"
            )
            nc.scalar.dma_start(
                w_stage[name][:], w_d[name].rearrange("(c2 p) co -> p c2 co", p=P)
            )
            w_sb[name] = consts.tile([P, C2, C], BF16, tag=f"w_{name}", name=f"w_{name}")
        # bq, bk as per-partition scalars [P, C2]; bv, bp as [1, C] rows.
        bqk_sb = {}
        for name in ("bq", "bk"):
            bqk_sb[name] = consts.tile([P, C2], F32, tag=f"b_{name}", name=f"b_{name}")
            nc.scalar.dma_start(
                bqk_sb[name][:], b_d[name].rearrange("(c2 p) -> p c2", p=P)
            )
        bv_row = consts.tile([1, C], F32)
        bp_row = consts.tile([1, C], F32)
        nc.scalar.dma_start(bv_row[:], b_d["bv"][None, :])
        nc.scalar.dma_start(bp_row[:], b_d["bp"][None, :])

        # ---- phase 1 stages -------------------------------------------
        xT = big.tile([P, C2, N], BF16, tag="xT")
        qT = big.tile([P, C2, N], BF16, tag="qT")
        kT = big.tile([P, C2, N], BF16, tag="kT")
        v_sb = big.tile([P, NK, C], BF16, tag="v")

        def stage_a(nb, dve_only=False):
            # transpose x rows for this 512-query block into xT
            # (copies out of the 2-deep misc PSUM ring; all-DVE for the
            # first two nb while ACT is still issuing weight DMAs).
            # transpose-mode doesn't count as HAM activity, so in the
            # head (dve_only) region a keep-warm matmul is woven after
            # each row-block - otherwise the PE re-throttles to 1.2 GHz
            # during the x-DMA-paced stretch and the first query blocks
            # of projections run at half clock.
            for j, nk in enumerate(range(4 * nb, 4 * nb + 4)):
                x_tile = x_tiles[nb][:, j]
                for c2 in range(C2):
                    tps = ps_misc.tile([P, P], F32, tag="misc")
                    # (float32r single-pass transpose was tried here: it
                    # passes CoreSim but crashes walrus/NEFF codegen.)
                    nc.tensor.transpose(
                        tps[:], x_tile[:, c2 * P : (c2 + 1) * P], identity[:]
                    )
                    dst_ap = xT[:, c2, nk * P : (nk + 1) * P]
                    if dve_only or c2 == 1:
                        nc.vector.tensor_copy(dst_ap, tps[:])
                    else:
                        nc.scalar.copy(dst_ap, tps[:])
                if dve_only:
                    warm_trickle(nb * 4 + j)

        def stage_b(nb):
            # v for the 4 nk chunks (stationary xT slices from stage_a(nb))
            for j, nk in enumerate(range(4 * nb, 4 * nb + 4)):
                pst = ps_proj.tile([P, C], F32, tag="mm_out")
                for ci2 in range(C2):
                    nc.tensor.matmul(
                        pst[:],
                        xT[:, ci2, nk * P : (nk + 1) * P],
                        w_sb["wv"][:, ci2, :],
                        start=(ci2 == 0),
                        stop=(ci2 == C2 - 1),
                    )
                # bv enters through bp_eff instead (attn rows sum to 1), so
                # v is the *raw* x@wv here.
                if j % 2 == 0:
                    nc.vector.tensor_copy(v_sb[:, nk, :], pst[:])
                else:
                    nc.scalar.copy(v_sb[:, nk, :], pst[:])
            # qT, kT blocks for this nb; bias-copies alternate ACT/DVE
            for dst, wname, bname, eng in (
                (qT, "wq", "bq", "act"),
                (kT, "wk", "bk", "dve"),
            ):
                for co2 in range(C2):
                    pst = ps_st.tile([P, QB], F32, tag="st")
                    for ci2 in range(C2):
                        nc.tensor.matmul(
                            pst[:],
                            w_sb[wname][:, ci2, co2 * P : (co2 + 1) * P],
                            xT[:, ci2, nb * QB : (nb + 1) * QB],
                            start=(ci2 == 0),
                            stop=(ci2 == C2 - 1),
                        )
                    dst_ap = dst[:, co2, nb * QB : (nb + 1) * QB]
                    bias_ap = bqk_sb[bname][:, co2 : co2 + 1]
                    if eng == "act":
                        nc.scalar.activation(dst_ap, pst[:], AF.Identity, bias=bias_ap)
                    else:
                        nc.vector.tensor_scalar_add(dst_ap, pst[:], bias_ap)

        bp_bcast = None

        def bias_prep():
            # bp_eff[co] = bp[co] + sum_c bv[c] wp[c, co]; broadcast [P, C].
            # First needed by the qb=0 epilogue precompute, ~60 us in.
            nonlocal bp_bcast
            bv_colps = ps_misc.tile([P, C2, 1], F32, tag="misc")
            for c2 in range(C2):
                nc.tensor.matmul(
                    bv_colps[:, c2],
                    bv_row[:, c2 * P : (c2 + 1) * P],
                    ones_col[:1],
                    start=True,
                    stop=True,
                )
            bv_col = consts.tile([P, C2, 1], F32)
            nc.vector.tensor_copy(bv_col[:], bv_colps[:])
            bvwp_ps = ps_misc.tile([1, C], F32, tag="misc")
            for c2 in range(C2):
                nc.tensor.matmul(
                    bvwp_ps[:],
                    bv_col[:, c2],
                    w_stage["wp"][:, c2, :],
                    start=(c2 == 0),
                    stop=(c2 == C2 - 1),
                )
            bp_eff_row = consts.tile([1, C], F32)
            nc.vector.tensor_add(bp_eff_row[:], bvwp_ps[:], bp_row[:])
            bpb_ps = ps_misc.tile([P, C], F32, tag="misc")
            nc.tensor.matmul(
                bpb_ps[:], ones_row[:], bp_eff_row[:], start=True, stop=True
            )
            bp_bcast = consts.tile([P, C], F32)
            nc.vector.tensor_copy(bp_bcast[:], bpb_ps[:])

        # ---- phase 1: stage_a two nb ahead of stage_b -----------------
        stage_a(0, dve_only=True)
        stage_a(1, dve_only=True)
        # Weight casts emitted here (DVE program order: after the first
        # transposes' copies, which they'd otherwise head-of-line block).
        # Casts on ACT: its DMA-issue chain ends ~13 us, right as the
        # staged weights land; DVE is still busy with transpose copies
        # until ~16 us, which used to gate stage_b(0) by ~3 us.
        nc.scalar.copy(w_sb["wv"][:], w_stage["wv"][:])
        nc.scalar.copy(w_sb["wq"][:], w_stage["wq"][:])
        nc.scalar.copy(w_sb["wk"][:], w_stage["wk"][:])

        # ---- attention: flat software-pipelined stream ----------------
        # Slot t emits: st-pair(t), raw-pair(t-2), one piece of boundary
        # work every other slot. The two-slot raw trail gives the exp
        # (~690 ns on ACT vs an 864 ns slot at full clock) a full slot of
        # latency slack; spacing the boundary extras keeps the proj ring
        # and DVE epilogue chain off the PE critical path.
        # Emitted as a GENERATOR: the first 28 slots of query-block 0 are
        # pumped into the phase-1 stream (4 after each stage_b group -
        # their kT/v chunks come from the previous group), so x-DMA and
        # copy waits in phase 1 are filled with ready attention work and
        # the kernel becomes one continuous PE stream.
        state = {}  # qb -> dict(rawT_ps, colsum, recip, rawT_sb, xbp)

        def st_pair(qb, mk):
            st_ps = ps_st.tile([P, QB], F32, tag="st")
            for ci2 in range(C2):
                nc.tensor.matmul(
                    st_ps[:],
                    kT[:, ci2, mk * P : (mk + 1) * P],
                    qT[:, ci2, qb * QB : (qb + 1) * QB],
                    start=(ci2 == 0),
                    stop=(ci2 == C2 - 1),
                )
            e_t = exp_pool.tile([P, QB], BF16, tag="eT")
            nc.scalar.activation(e_t[:], st_ps[:], AF.Exp, scale=SCALE)
            return e_t

        def raw_pair(qb, mk, e_t):
            st_ = state.setdefault(qb, {})
            if mk == 0:
                st_["rawT_ps"] = ps_raw.tile(
                    [P, C2, QB], F32, tag="rawT", name=f"rawT_{qb}"
                )
                st_["colsum"] = sums.tile(
                    [P, QB], F32, tag="colsum", name=f"colsum_{qb}"
                )
            rawT_ps = st_["rawT_ps"]
            colsum = st_["colsum"]
            for c2 in range(C2):
                nc.tensor.matmul(
                    rawT_ps[:, c2],
                    v_sb[:, mk, c2 * P : (c2 + 1) * P],
                    e_t[:],
                    start=(mk == 0),
                    stop=(mk == NK - 1),
                )
            if mk == 0:
                nc.vector.tensor_copy(colsum[:], e_t[:])
            else:
                nc.vector.tensor_add(colsum[:], colsum[:], e_t[:])
            if mk == NK - 1:
                # rawT -> SBUF bf16 for the proj matmuls. Both halves on
                # DVE: putting them on ACT delays the exp cadence (ACT
                # runs at ~80% of a slot on exps alone), which stalls the
                # score ring two slots later; DVE has boundary slack.
                rawT_sb = rawsb.tile(
                    [P, C2, QB], BF16, tag="rawT_sb", name=f"rawT_sb_{qb}"
                )
                if qb == NQB - 1:
                    # last block: exp stream is over, ACT is idle - split
                    # the copies so the tail chain starts ~0.5 us earlier
                    nc.scalar.copy(rawT_sb[:, 0], rawT_ps[:, 0])
                else:
                    nc.vector.tensor_copy(rawT_sb[:, 0], rawT_ps[:, 0])
                nc.vector.tensor_copy(rawT_sb[:, 1], rawT_ps[:, 1])
                st_["rawT_sb"] = rawT_sb
                return True
            return False

        def extra_den(qb):
            st_ = state[qb]
            den_ps = ps_misc.tile([P, QSUB], F32, tag="misc")
            for qs in range(QSUB):
                nc.tensor.matmul(
                    den_ps[:, qs : qs + 1],
                    st_["colsum"][:, qs * P : (qs + 1) * P],
                    ones_col[:],
                    start=True,
                    stop=True,
                )
            recip = sums.tile([P, QSUB], F32, tag="recip")
            nc.vector.reciprocal(recip[:], den_ps[:])
            st_["recip"] = recip

        def extra_proj(qb, qs):
            st_ = state[qb]
            pj_ps = ps_proj.tile([P, C], F32, tag="mm_out", name=f"pj_{qb}_{qs}")
            for c2 in range(C2):
                nc.tensor.matmul(
                    pj_ps[:],
                    st_["rawT_sb"][:, c2, qs * P : (qs + 1) * P],
                    w_sb["wp"][:, c2, :],
                    start=(c2 == 0),
                    stop=(c2 == C2 - 1),
                )
            if qs == 0:
                st_["o_big"] = epil.tile(
                    [P, QSUB, C], F32, tag="o_big", name=f"o_{qb}", bufs=2
                )
            o_ap = st_["o_big"][:, qs]
            last_qb = qb == NQB - 1
            # o = proj * recip[q] + (x + bp_eff). Mid-kernel both ops run
            # on DVE (ACT is exp-saturated); for the final block the mul
            # moves to the now-idle ACT and each row-block DMAs out as
            # soon as its add lands, so the tail pipelines.
            if last_qb and qs % 2 == 0:
                # alternate engines so consecutive proj ring slots drain
                # in parallel in the dense tail
                nc.scalar.activation(
                    o_ap, pj_ps[:], AF.Identity, scale=st_["recip"][:, qs : qs + 1]
                )
            else:
                nc.vector.tensor_scalar_mul(
                    o_ap, pj_ps[:], st_["recip"][:, qs : qs + 1]
                )
            nc.vector.tensor_add(o_ap, o_ap, st_["xbp"][:, qs])
            if last_qb:
                n0 = qb * QB + qs * P
                nc.sync.dma_start(out_d[n0 : n0 + P, :], o_ap)
                if qs == QSUB - 1:
                    del state[qb]
            elif qs == QSUB - 1:
                n0 = qb * QB
                nc.sync.dma_start(
                    out_d[n0 : n0 + QB, :].rearrange("(t p) c -> p t c", p=P),
                    st_["o_big"][:],
                )
                del state[qb]

        def emit_xbp(qb, qs):
            # xbp = x + bp_bcast for sub-block qs of qb (DVE, early)
            st_ = state.setdefault(qb, {})
            if qs == 0:
                st_["xbp"] = epil.tile(
                    [P, QSUB, C], F32, tag="xbp", name=f"xbp_{qb}", bufs=2
                )
            nc.vector.tensor_add(
                st_["xbp"][:, qs], x_tiles[qb][:, qs], bp_bcast[:]
            )

        def attention_slots():
            pending = []  # (qb, mk, e_t) raws not yet issued (2-slot trail)
            extras = []  # deferred boundary closures, one per odd slot
            arm = None  # qb whose extras to queue at the end of this slot
            for qb in range(NQB):
                for mk in range(NK):
                    e_t = st_pair(qb, mk)
                    pending.append((qb, mk, e_t))
                    # raw stream trails by 2 slots; skip one slot right
                    # after a boundary so the rawT PSUM drain never stalls.
                    target = 3 if (qb > 0 and mk == 2) else 2
                    while len(pending) > target:
                        item = pending.pop(0)
                        if raw_pair(*item):
                            arm = item[0]
                    # extras every THIRD slot: den@3, proj@6,9,12,15 -
                    # their DVE mul/add on top of the ~690 ns colsum was
                    # over the 864 ns slot budget at every-other-slot
                    # spacing, and the accumulated DVE lag reached the
                    # exp's e_t ring guard.
                    if extras and mk >= 3 and mk % 3 == 0:
                        extras.pop(0)()
                    elif 17 <= mk < 21:
                        emit_xbp(qb, mk - 17)
                    if arm is not None:
                        extras = [lambda qb=arm: extra_den(qb)] + [
                            lambda qb=arm, qs=qs: extra_proj(qb, qs)
                            for qs in range(QSUB)
                        ]
                        arm = None
                    yield
            while pending:
                raw_pair(*pending.pop(0))
            extras = [lambda: extra_den(NQB - 1)] + [
                lambda qs=qs: extra_proj(NQB - 1, qs) for qs in range(QSUB)
            ]
            while extras:
                extras.pop(0)()

        # ---- phase 1 + fused attention start --------------------------
        att = attention_slots()
        for nb in range(NQB):
            stage_b(nb)
            if nb == 2:
                nc.scalar.copy(w_sb["wp"][:], w_stage["wp"][:])
                bias_prep()
            if nb >= 1:
                # pump 4 attention slots (chunks of the previous group)
                # BEFORE the next transposes: ready PE work that covers
                # the x-DMA arrival window for stage_a(nb + 2).
                for _ in range(4):
                    next(att, None)
            if nb + 2 < NQB:
                stage_a(nb + 2)
        for _ in att:
            pass


def kernel(**inputs):
    global _CACHED_NC
    if _CACHED_NC is None:
        _CACHED_NC = _build()
    nc = _CACHED_NC

    x = np.ascontiguousarray(inputs["x"], dtype=np.float32)  # [B, H, W, C]
    shared = {
        name: np.ascontiguousarray(inputs[name], dtype=np.float32)
        for name in ("wq", "bq", "wk", "bk", "wv", "bv", "wp", "bp")
    }
    in_maps = [
        {"x": x[b].reshape(N, C), **shared} for b in range(B)
    ]
    res = bass_utils.run_bass_kernel_spmd(nc, in_maps, core_ids=list(range(B)))
    out = np.stack([res.results[b]["out"] for b in range(B)], axis=0)
    return out.reshape(B, H, W, C)
